# revision 1
# baseline (speedup 1.0000x reference)
"""Multi-head attention (softmax over the QUERY axis) on 8 TRN2 NeuronCores.

Sharding: 2 batches x 4 head-groups (4 heads each) -> 8 cores.
Each core computes, for its (batch b, heads 4g..4g+3):
    qkT = W_{q,k} @ x_b^T + b_{q,k}   [512, 2048]   (e_out on partitions)
    V   = x_b @ W_v^T + b_v           [2048, 256]
    S'  = K Q^T (scores TRANSPOSED)   [k, q] per head
    P   = exp(S'/8) with fused row-sum -> denom[k]  (softmax over q == free dim)
    outT= sum_k (V[k,:]/denom[k]) P[k,:]            [d, q] per head
    part= outT^T @ WoT_g              [2048, 1024]  (partial for this head group)
Host sums the 4 partials per batch and adds bo (the tensor-parallel epilogue).

Matmul inputs are bf16 (PSUM accumulation, softmax statistics and V'/denom
math stay fp32); host pre-casts x/W to bf16 (halves input DMA). Head pairs
share the PE array via disjoint row groups (scores: d at partitions 0/64)
and disjoint column groups (attn.V: outT partitions 0/64).

Pipelining: only the Q/K tiles for head-pair 0 are computed up front; the
remaining qkT/V work is emitted as PE "filler" groups interleaved into
pair 0's attention k-loop, so the PE never idles while ACT paces the
exp stream. attn.V accumulates in a 2-bank transient PSUM tile per
(4-ktile group, q-half) and flushes into an SBUF fp32 accumulator, keeping
total PSUM at 8 banks: S'(2x2) + attn.V(2) + qkv/final groups(2).
"""

import sys

if "/opt/trn_rl_repo" not in sys.path:
    sys.path.insert(0, "/opt/trn_rl_repo")

import numpy as np
import ml_dtypes

import concourse.bass as bass
import concourse.mybir as mybir
import concourse.tile as tile
from concourse import bacc
from concourse.bass_utils import run_bass_kernel_spmd

F32 = mybir.dt.float32
BF16 = mybir.dt.bfloat16
AF = mybir.ActivationFunctionType

B, S, E, H = 2, 2048, 1024, 16
HL = 4  # heads per core
DH = 64
QK = 512  # q+k out dims per core (2*HL*DH)
V3 = 768  # q+k+v out dims per core
NCORES = 8

ET = E // 128  # 8 e-tiles
ST = S // 128  # 16 s-tiles
SC = S // 512  # 4 s/q chunks of 512
KT = ST  # 16 k-tiles
FG = 4  # k-tiles per attn.V accumulation group

LAST_RESULTS = None


def build_kernel():
    nc = bacc.Bacc("TRN2", target_bir_lowering=False, debug=False, num_devices=NCORES)

    xT = nc.dram_tensor("xT", [E, S], BF16, kind="ExternalInput")
    wT = nc.dram_tensor("wT", [E, V3], BF16, kind="ExternalInput")
    bq = nc.dram_tensor("bq", [128, 4], F32, kind="ExternalInput")
    bv = nc.dram_tensor("bv", [1, 256], BF16, kind="ExternalInput")
    woT = nc.dram_tensor("woT", [2 * 128, E], BF16, kind="ExternalInput")
    out0 = nc.dram_tensor("out0", [S, E], F32, kind="ExternalOutput")
    out1 = nc.dram_tensor("out1", [S, E], F32, kind="ExternalOutput")

    with tile.TileContext(nc) as tc:
        with (
            tc.tile_pool(name="persist", bufs=1) as persist,
            tc.tile_pool(name="smalls", bufs=3) as smalls,
            tc.tile_pool(name="expp", bufs=2 * FG) as expp,
            tc.tile_pool(name="vsp", bufs=2 * FG + 2) as vsp,
            tc.tile_pool(name="fout", bufs=2) as foutp,
            tc.tile_pool(name="mm_ps", bufs=2, space="PSUM") as mm_ps,
            tc.tile_pool(name="sp_ps", bufs=2, space="PSUM") as sp_ps,
            tc.tile_pool(name="ot_ps", bufs=1, space="PSUM") as ot_ps,
        ):
            qk_sb = persist.tile([128, 4, S], BF16, tag="qk")
            v_sb = persist.tile([128, ST, 256], F32, tag="v")
            outT_f32 = persist.tile([128, 2, S], F32, tag="outT")
            outT_bf = persist.tile([128, 2, S], BF16, tag="outT_bf")
            bq_sb = persist.tile([128, 4], F32, tag="bq")
            bv_sb = persist.tile([1, 256], BF16, tag="bv")
            ones_sb = persist.tile([1, 512], BF16, tag="ones")
            xt_sb = persist.tile([128, ET, S], BF16, tag="xt")
            wt_sb = persist.tile([128, ET, V3], BF16, tag="wt")
            wo_sb = persist.tile([128, 2, E], BF16, tag="wo")

            nc.vector.memset(ones_sb[:], 1.0)
            for et in range(ET):
                nc.gpsimd.dma_start(wt_sb[:, et, :], wT[et * 128 : (et + 1) * 128, :])
            for sc in range(SC):
                for et in range(ET):
                    nc.sync.dma_start(
                        xt_sb[:, et, sc * 512 : (sc + 1) * 512],
                        xT[et * 128 : (et + 1) * 128, sc * 512 : (sc + 1) * 512],
                    )
            nc.gpsimd.dma_start(bq_sb[:], bq[:])
            nc.gpsimd.dma_start(bv_sb[:], bv[:])
            for p in range(2):
                nc.gpsimd.dma_start(wo_sb[:, p, :], woT[p * 128 : (p + 1) * 128, :])

            # ---- emitters for qkT / V accumulation groups ----------------
            def emit_qk_group(eo, sc):
                pt = mm_ps.tile([128, 512], F32, tag="mmps")
                for et in range(ET):
                    nc.tensor.matmul(
                        pt[:],
                        wt_sb[:, et, eo * 128 : (eo + 1) * 128],
                        xt_sb[:, et, sc * 512 : (sc + 1) * 512],
                        start=(et == 0),
                        stop=(et == ET - 1),
                    )
                nc.vector.tensor_scalar_add(
                    qk_sb[:, eo, sc * 512 : (sc + 1) * 512],
                    in0=pt[:],
                    scalar1=bq_sb[:, eo : eo + 1],
                )

            def emit_v_group(st):
                pt = mm_ps.tile([128, 512], F32, tag="mmps")
                for et in range(ET):
                    nc.tensor.matmul(
                        pt[:, :256],
                        xt_sb[:, et, st * 128 : (st + 1) * 128],
                        wt_sb[:, et, QK:V3],
                        start=(et == 0),
                        stop=False,
                    )
                nc.tensor.matmul(  # + ones^T bv (bias row)
                    pt[:, :256],
                    ones_sb[0:1, 0:128],
                    bv_sb[0:1, :],
                    start=False,
                    stop=True,
                )
                nc.vector.tensor_copy(v_sb[:, st, :], pt[:, :256])

            def emit_d_group(p, st, out_dram):
                ot = foutp.tile([128, E], F32, tag="fout", name=f"fo_{p}_{st}")
                for nck in range(2):
                    pt = mm_ps.tile([128, 512], F32, tag="mmps", name=f"fp_{p}_{st}_{nck}")
                    nc.tensor.matmul(
                        pt[:],
                        outT_bf[:, p, st * 128 : (st + 1) * 128],
                        wo_sb[:, p, nck * 512 : (nck + 1) * 512],
                        start=True,
                        stop=True,
                    )
                    if p == 1 and nck == 1:
                        nc.scalar.copy(ot[:, nck * 512 : (nck + 1) * 512], pt[:])
                    else:
                        nc.vector.tensor_copy(ot[:, nck * 512 : (nck + 1) * 512], pt[:])
                nc.sync.dma_start(out_dram[st * 128 : (st + 1) * 128, :], ot[:])

            # ---- pre-attention: just enough for pair0 kt0 ----------------
            # Emission order IS program order: every filler must be emitted
            # no later than the k-tile iteration that first consumes it
            # (fillers pop at the TOP of each k-tile iteration).
            emit_qk_group(0, 0)  # Q heads 0,1 cols 0-511
            emit_qk_group(0, 1)
            emit_qk_group(2, 0)  # K heads 0,1 cols 0-511 (kts 0-3)

            def qg(eo, sc):
                return lambda: emit_qk_group(eo, sc)

            def vg(st):
                return lambda: emit_v_group(st)

            fillers = (
                [vg(0), vg(1), qg(2, 1), vg(2), vg(3), qg(2, 2), vg(4), qg(2, 3)]
                + [vg(5), vg(6), vg(7), vg(8)]
                + [qg(1, 0), qg(1, 1), qg(1, 2), qg(1, 3)]
                + [vg(9), vg(10)]
                + [qg(3, 0), qg(3, 1)]
                + [vg(11), vg(12), vg(13), vg(14), vg(15)]
            )
            fillers.reverse()  # pop() from the front

            # ---- attention per head pair ---------------------------------
            # attn.V slices for group g are spread over group g+1's k-tiles
            # (2 of a half's 4 j-steps per k-tile) so the PE load per k-tile
            # is even and the exp stream never sees a burst.
            c_state = {}

            def emit_c_slices(p, g, half, jpair, exs, vss):
                if jpair == 0:
                    c_state[half] = ot_ps.tile(
                        [128, 1024], F32, tag="otps", name=f"oTt_{p}_{g}_{half}"
                    )
                oTt = c_state[half]
                for j in (2 * jpair, 2 * jpair + 1):
                    kt = FG * g + j
                    for hh in range(2):
                        for qc in range(2):
                            q0 = half * 1024 + qc * 512
                            nc.tensor.matmul(
                                oTt[
                                    hh * 64 : (hh + 1) * 64,
                                    qc * 512 : (qc + 1) * 512,
                                ],
                                vss[kt][:, hh, :],
                                exs[kt][:, hh, q0 : q0 + 512],
                                start=(j == 0),
                                stop=(j == FG - 1),
                            )
                if jpair == 1:
                    dst = outT_f32[:, p, half * 1024 : (half + 1) * 1024]
                    if g == 0:
                        nc.vector.tensor_copy(dst, oTt[:])
                    else:
                        nc.vector.tensor_add(dst, dst, oTt[:])

            for p in range(2):
                exs = {}
                vss = {}
                for kt in range(KT):
                    ex = expp.tile([128, 2, S], BF16, tag="exp")
                    exs[kt] = ex
                    den = smalls.tile([128, 2, 2], F32, tag="den")
                    for half in range(2):
                        for hh in range(2):
                            sp = sp_ps.tile([128, 1024], F32, tag="sp")
                            for qc in range(2):
                                q0 = half * 1024 + qc * 512
                                nc.tensor.matmul(
                                    sp[:, qc * 512 : (qc + 1) * 512],
                                    qk_sb[
                                        hh * 64 : (hh + 1) * 64,
                                        2 + p,
                                        kt * 128 : (kt + 1) * 128,
                                    ],
                                    qk_sb[hh * 64 : (hh + 1) * 64, p, q0 : q0 + 512],
                                    start=True,
                                    stop=True,
                                )
                            nc.scalar.activation(
                                ex[:, hh, half * 1024 : (half + 1) * 1024],
                                sp[:],
                                AF.Exp,
                                scale=0.125,
                                accum_out=den[:, hh, half : half + 1],
                            )
                        if p == 0 and kt == 0 and half == 0:
                            emit_qk_group(0, 2)  # Q cols 1024-2047 for half1
                            emit_qk_group(0, 3)
                    # previous group's attn.V, 8 matmuls per k-tile
                    if kt >= FG:
                        o = kt % FG
                        emit_c_slices(p, kt // FG - 1, o // 2, o % 2, exs, vss)
                    # PE fillers (producers before their consumers)
                    if p == 0:
                        for _ in range(2):
                            if fillers:
                                fillers.pop()()
                    elif kt < 2:  # pair1 kt0/1: remaining K tiles for heads 2,3
                        emit_qk_group(3, 2 + kt)
                    else:  # pair1: overlap pair0's projection
                        emit_d_group(0, kt - 2, out0)
                        if kt >= 14:
                            emit_d_group(0, kt - 2 + 2, out0)
                    dsum = smalls.tile([128, 2], F32, tag="dsum")
                    nc.vector.tensor_add(dsum[:], den[:, :, 0], den[:, :, 1])
                    rec = smalls.tile([128, 2], F32, tag="rec")
                    nc.vector.reciprocal(rec[:], dsum[:])
                    vs = vsp.tile([128, 2, DH], BF16, tag="vs")
                    vss[kt] = vs
                    for hh in range(2):
                        nc.vector.tensor_scalar_mul(
                            vs[:, hh, :],
                            in0=v_sb[:, kt, (2 * p + hh) * 64 : (2 * p + hh + 1) * 64],
                            scalar1=rec[:, hh : hh + 1],
                        )
                # tail: last group's attn.V (both q-halves), then the
                # projection; copies split across DVE and the idle ACT
                for half in range(2):
                    emit_c_slices(p, KT // FG - 1, half, 0, exs, vss)
                    emit_c_slices(p, KT // FG - 1, half, 1, exs, vss)
                    nc.vector.tensor_copy(
                        outT_bf[:, p, half * 1024 : (half + 1) * 1024],
                        outT_f32[:, p, half * 1024 : (half + 1) * 1024],
                    )
                if p == 1:
                    for st in range(ST):
                        emit_d_group(1, st, out1)


    nc.compile()
    return nc


def _shard_inputs(input, Wqkv, bqkv, Wo):
    """Build the 8 per-core input dicts (host-side layout/sharding)."""
    bf16 = ml_dtypes.bfloat16
    in_maps = []
    for c in range(NCORES):
        b = c // 4
        g = c % 4
        heads = range(4 * g, 4 * g + 4)
        rows = (
            [slice(64 * h, 64 * h + 64) for h in heads]
            + [slice(E + 64 * h, E + 64 * h + 64) for h in heads]
            + [slice(2 * E + 64 * h, 2 * E + 64 * h + 64) for h in heads]
        )
        W_sel = np.concatenate([Wqkv[s] for s in rows], axis=0)  # [768, 1024]
        b_sel = np.concatenate([bqkv[s] for s in rows], axis=0)  # [768]
        in_maps.append(
            {
                "xT": np.ascontiguousarray(input[b].T).astype(bf16),
                "wT": np.ascontiguousarray(W_sel.T).astype(bf16),
                "bq": np.ascontiguousarray(b_sel[:QK].reshape(4, 128).T),
                "bv": np.ascontiguousarray(b_sel[QK:V3].reshape(1, 256)).astype(bf16),
                "woT": np.ascontiguousarray(
                    Wo[:, 4 * g * DH : 4 * (g + 1) * DH].T
                ).astype(bf16),
            }
        )
    return in_maps


def kernel(input, Wqkv, bqkv, Wo, bo, _trace=False):
    global LAST_RESULTS
    input = np.asarray(input, dtype=np.float32)
    Wqkv = np.asarray(Wqkv, dtype=np.float32)
    bqkv = np.asarray(bqkv, dtype=np.float32)
    Wo = np.asarray(Wo, dtype=np.float32)
    bo = np.asarray(bo, dtype=np.float32)

    nc = build_kernel()
    in_maps = _shard_inputs(input, Wqkv, bqkv, Wo)
    kwargs = {}
    if _trace:
        kwargs = dict(trace=True, trace_cores=[0])
    res = run_bass_kernel_spmd(nc, in_maps, core_ids=list(range(NCORES)), **kwargs)
    LAST_RESULTS = res

    out = np.zeros((B, S, E), dtype=np.float32)
    for c in range(NCORES):
        out[c // 4] += res.results[c]["out0"]
        out[c // 4] += res.results[c]["out1"]
    out += bo
    return out



# revision 2
# speedup vs baseline: 1.0939x; 1.0939x over previous
"""Multi-head attention (softmax over the QUERY axis) on 8 TRN2 NeuronCores.

Sharding: 2 batches x 4 head-groups (4 heads each) -> 8 cores.
Each core computes, for its (batch b, heads 4g..4g+3):
    qkT = W_{q,k} @ x_b^T + b_{q,k}   [512, 2048]   (e_out on partitions)
    V   = x_b @ W_v^T + b_v           [2048, 256]
    S'  = K Q^T (scores TRANSPOSED)   [k, q] per head
    P   = exp(S'/8) with fused row-sum -> denom[k]  (softmax over q == free dim)
    outT= sum_k (V[k,:]/denom[k]) P[k,:]            [d, q] per head
    part= outT^T @ WoT_g              [2048, 1024]  (partial for this head group)
Host sums the partials per batch (fp32) and adds bo.

Perf structure (v2):
  - A dense block of dummy PE matmuls at t=0 warms the HAM clock gate
    (PE 1.2 -> 2.4 GHz) while the input DMAs land, so all real matmuls
    run at the warm clock.
  - ACT (the only exp engine) is the roofline: 128 exp calls of
    [128,1024] + fused accumulator reads.  Everything else (PE matmul
    stream, DVE PSUM drains, DMA) is packed around that stream.
  - V is stored bf16 so the per-k V/denom scaling runs in DVE 4x mode.
  - Denominator bookkeeping (half-sum + reciprocal) is batched per
    4-ktile group into persistent tiles.
  - attn.V accumulates per 4-ktile group in PSUM; group flushes go
    straight into the SBUF fp32 accumulator, the last group's flush
    emits bf16 directly (no separate cast pass).
  - Projection outputs are written as bf16 partials; the p1 tail
    alternates its PSUM->SBUF copies between DVE and the idle ACT and
    interleaves the half0 projection with half1's attn.V tail.
"""

import sys

if "/opt/trn_rl_repo" not in sys.path:
    sys.path.insert(0, "/opt/trn_rl_repo")

import numpy as np
import ml_dtypes

import concourse.bass as bass
import concourse.mybir as mybir
import concourse.tile as tile
from concourse import bacc
from concourse.bass_utils import run_bass_kernel_spmd

F32 = mybir.dt.float32
BF16 = mybir.dt.bfloat16
AF = mybir.ActivationFunctionType

B, S, E, H = 2, 2048, 1024, 16
HL = 4  # heads per core
DH = 64
QK = 512  # q+k out dims per core (2*HL*DH)
V3 = 768  # q+k+v out dims per core
NCORES = 8

ET = E // 128  # 8 e-tiles
ST = S // 128  # 16 s-tiles
SC = S // 512  # 4 s/q chunks of 512
KT = ST  # 16 k-tiles
FG = 4  # k-tiles per attn.V accumulation group
NWARM = 24  # dummy matmuls to warm the PE clock gate

LAST_RESULTS = None


def build_kernel():
    nc = bacc.Bacc("TRN2", target_bir_lowering=False, debug=False, num_devices=NCORES)

    xT = nc.dram_tensor("xT", [E, S], BF16, kind="ExternalInput")
    wT = nc.dram_tensor("wT", [E, V3], BF16, kind="ExternalInput")
    bq = nc.dram_tensor("bq", [128, 4], F32, kind="ExternalInput")
    bv = nc.dram_tensor("bv", [1, 256], BF16, kind="ExternalInput")
    woT = nc.dram_tensor("woT", [2 * 128, E], BF16, kind="ExternalInput")
    out0 = nc.dram_tensor("out0", [S, E], BF16, kind="ExternalOutput")
    out1 = nc.dram_tensor("out1", [S, E], BF16, kind="ExternalOutput")

    with tile.TileContext(nc) as tc:
        with (
            tc.tile_pool(name="persist", bufs=1) as persist,
            tc.tile_pool(name="smalls", bufs=3) as smalls,
            tc.tile_pool(name="expp", bufs=2 * FG) as expp,
            tc.tile_pool(name="vsp", bufs=3) as vsp,
            tc.tile_pool(name="fout", bufs=2) as foutp,
            tc.tile_pool(name="mm_ps", bufs=2, space="PSUM") as mm_ps,
            tc.tile_pool(name="sp_ps", bufs=2, space="PSUM") as sp_ps,
            tc.tile_pool(name="ot_ps", bufs=1, space="PSUM") as ot_ps,
        ):
            qk_sb = persist.tile([128, 4, S], BF16, tag="qk")
            v_sb = persist.tile([128, ST, 256], BF16, tag="v")
            outT_f32 = persist.tile([128, 2, S], F32, tag="outT")
            outT_bf = persist.tile([128, 2, S], BF16, tag="outT_bf")
            bq_sb = persist.tile([128, 4], F32, tag="bq")
            bv_sb = persist.tile([1, 256], BF16, tag="bv")
            ones_sb = persist.tile([1, 512], BF16, tag="ones")
            den_sb = persist.tile([128, KT, 2, 2], F32, tag="den")
            xt_sb = persist.tile([128, ET, S], BF16, tag="xt")
            wt_sb = persist.tile([128, ET, V3], BF16, tag="wt")
            wo_sb = persist.tile([128, 2, E], BF16, tag="wo")

            # ---- PE warm-up: dense dummy matmuls while DMAs land ---------
            nc.vector.memset(ones_sb[:], 1.0)
            for _ in range(NWARM):
                wp = mm_ps.tile([128, 512], F32, tag="mmps")
                nc.tensor.matmul(
                    wp[:], ones_sb[0:1, 0:128], ones_sb[0:1, 0:512],
                    start=True, stop=True,
                )

            for et in range(ET):
                nc.gpsimd.dma_start(wt_sb[:, et, :], wT[et * 128 : (et + 1) * 128, :])
            for sc in range(SC):
                for et in range(ET):
                    nc.sync.dma_start(
                        xt_sb[:, et, sc * 512 : (sc + 1) * 512],
                        xT[et * 128 : (et + 1) * 128, sc * 512 : (sc + 1) * 512],
                    )
            nc.gpsimd.dma_start(bq_sb[:], bq[:])
            nc.gpsimd.dma_start(bv_sb[:], bv[:])
            for p in range(2):
                nc.gpsimd.dma_start(wo_sb[:, p, :], woT[p * 128 : (p + 1) * 128, :])

            # ---- emitters for qkT / V accumulation groups ----------------
            def emit_qk_group(eo, sc):
                pt = mm_ps.tile([128, 512], F32, tag="mmps")
                for et in range(ET):
                    nc.tensor.matmul(
                        pt[:],
                        wt_sb[:, et, eo * 128 : (eo + 1) * 128],
                        xt_sb[:, et, sc * 512 : (sc + 1) * 512],
                        start=(et == 0),
                        stop=(et == ET - 1),
                    )
                nc.vector.tensor_scalar_add(
                    qk_sb[:, eo, sc * 512 : (sc + 1) * 512],
                    in0=pt[:],
                    scalar1=bq_sb[:, eo : eo + 1],
                )

            def emit_v_group(st):
                pt = mm_ps.tile([128, 512], F32, tag="mmps")
                for et in range(ET):
                    nc.tensor.matmul(
                        pt[:, :256],
                        xt_sb[:, et, st * 128 : (st + 1) * 128],
                        wt_sb[:, et, QK:V3],
                        start=(et == 0),
                        stop=False,
                    )
                nc.tensor.matmul(  # + ones^T bv (bias row)
                    pt[:, :256],
                    ones_sb[0:1, 0:128],
                    bv_sb[0:1, :],
                    start=False,
                    stop=True,
                )
                nc.vector.tensor_copy(v_sb[:, st, :], pt[:, :256])

            def emit_d_group(p, st, out_dram, tail=False):
                ot = foutp.tile([128, E], BF16, tag="fout", name=f"fo_{p}_{st}")
                for nck in range(2):
                    pt = mm_ps.tile([128, 512], F32, tag="mmps", name=f"fp_{p}_{st}_{nck}")
                    nc.tensor.matmul(
                        pt[:],
                        outT_bf[:, p, st * 128 : (st + 1) * 128],
                        wo_sb[:, p, nck * 512 : (nck + 1) * 512],
                        start=True,
                        stop=True,
                    )
                    if tail and nck == 1:
                        nc.scalar.copy(ot[:, nck * 512 : (nck + 1) * 512], pt[:])
                    else:
                        nc.vector.tensor_copy(ot[:, nck * 512 : (nck + 1) * 512], pt[:])
                nc.sync.dma_start(out_dram[st * 128 : (st + 1) * 128, :], ot[:])

            # ---- pre-attention: just enough for pair0 kt0 ----------------
            # Emission order IS program order: every filler must be emitted
            # no later than the k-tile iteration that first consumes it
            # (fillers pop at the TOP of each k-tile iteration).
            emit_qk_group(0, 0)  # Q heads 0,1 cols 0-511
            emit_qk_group(0, 1)
            emit_qk_group(2, 0)  # K heads 0,1 cols 0-511 (kts 0-3)

            def qg(eo, sc):
                return lambda: emit_qk_group(eo, sc)

            def vg(st):
                return lambda: emit_v_group(st)

            fillers = (
                [vg(0), vg(1), qg(2, 1), vg(2), vg(3), qg(2, 2), vg(4), qg(2, 3)]
                + [vg(5), vg(6), vg(7), vg(8)]
                + [qg(1, 0), qg(1, 1), qg(1, 2), qg(1, 3)]
                + [vg(9), vg(10)]
                + [qg(3, 0), qg(3, 1)]
                + [vg(11), vg(12), vg(13), vg(14), vg(15)]
            )
            fillers.reverse()  # pop() from the front

            # ---- attention per head pair ---------------------------------
            # attn.V slices for group g are spread over group g+1's k-tiles
            # (2 of a half's 4 j-steps per k-tile) so the PE load per k-tile
            # is even and the exp stream never sees a burst.
            c_state = {}

            def emit_c_slices(p, g, half, jpair, exs, vss):
                if jpair == 0:
                    c_state[half] = ot_ps.tile(
                        [128, 1024], F32, tag="otps", name=f"oTt_{p}_{g}_{half}"
                    )
                oTt = c_state[half]
                for j in (2 * jpair, 2 * jpair + 1):
                    kt = FG * g + j
                    vs_g, jj = vss[kt]
                    for hh in range(2):
                        for qc in range(2):
                            q0 = half * 1024 + qc * 512
                            nc.tensor.matmul(
                                oTt[
                                    hh * 64 : (hh + 1) * 64,
                                    qc * 512 : (qc + 1) * 512,
                                ],
                                vs_g[:, jj, hh, :],
                                exs[kt][:, hh, q0 : q0 + 512],
                                start=(j == 0),
                                stop=(j == FG - 1),
                            )
                if jpair == 1:
                    f32dst = outT_f32[:, p, half * 1024 : (half + 1) * 1024]
                    if g == 0:
                        nc.vector.tensor_copy(f32dst, oTt[:])
                    elif g < KT // FG - 1:
                        nc.vector.tensor_add(f32dst, f32dst, oTt[:])
                    else:  # final group: emit bf16 directly
                        nc.vector.tensor_add(
                            outT_bf[:, p, half * 1024 : (half + 1) * 1024],
                            f32dst,
                            oTt[:],
                        )

            for p in range(2):
                exs = {}
                vss = {}
                for kt in range(KT):
                    ex = expp.tile([128, 2, S], BF16, tag="exp")
                    exs[kt] = ex
                    for half in range(2):
                        for hh in range(2):
                            sp = sp_ps.tile([128, 1024], F32, tag="sp")
                            for qc in range(2):
                                q0 = half * 1024 + qc * 512
                                nc.tensor.matmul(
                                    sp[:, qc * 512 : (qc + 1) * 512],
                                    qk_sb[
                                        hh * 64 : (hh + 1) * 64,
                                        2 + p,
                                        kt * 128 : (kt + 1) * 128,
                                    ],
                                    qk_sb[hh * 64 : (hh + 1) * 64, p, q0 : q0 + 512],
                                    start=True,
                                    stop=True,
                                )
                            nc.scalar.activation(
                                ex[:, hh, half * 1024 : (half + 1) * 1024],
                                sp[:],
                                AF.Exp,
                                scale=0.125,
                                accum_out=den_sb[:, kt, hh, half : half + 1],
                            )
                        if p == 0 and kt == 0 and half == 0:
                            emit_qk_group(0, 2)  # Q cols 1024-2047 for half1
                            emit_qk_group(0, 3)
                    # previous group's attn.V, 8 matmuls per k-tile
                    if kt >= FG:
                        o = kt % FG
                        emit_c_slices(p, kt // FG - 1, o // 2, o % 2, exs, vss)
                    # PE fillers (producers before their consumers)
                    if p == 0:
                        for _ in range(2):
                            if fillers:
                                fillers.pop()()
                    elif kt < 2:  # pair1 kt0/1: remaining K tiles for heads 2,3
                        emit_qk_group(3, 2 + kt)
                    else:  # pair1: overlap pair0's projection
                        emit_d_group(0, kt - 2, out0)
                        if kt >= 14:
                            emit_d_group(0, kt - 2 + 2, out0)
                    # batched denominator bookkeeping per 4-ktile group
                    if kt % 4 == 3:
                        k0 = kt - 3
                        dsum = smalls.tile([128, 4, 2], F32, tag="dsum")
                        nc.vector.tensor_add(
                            dsum[:], den_sb[:, k0 : k0 + 4, :, 0], den_sb[:, k0 : k0 + 4, :, 1]
                        )
                        rec = smalls.tile([128, 4, 2], F32, tag="rec")
                        nc.vector.reciprocal(rec[:], dsum[:])
                        vs_g = vsp.tile([128, 4, 2, DH], BF16, tag="vs")
                        for j in range(4):
                            vss[k0 + j] = (vs_g, j)
                            for hh in range(2):
                                nc.vector.tensor_scalar_mul(
                                    vs_g[:, j, hh, :],
                                    in0=v_sb[:, k0 + j, (2 * p + hh) * 64 : (2 * p + hh + 1) * 64],
                                    scalar1=rec[:, j, hh : hh + 1],
                                )
                # tail: last group's attn.V (both q-halves) + flush; for p1
                # interleave the half0 projection with half1's attn.V tail
                for half in range(2):
                    emit_c_slices(p, KT // FG - 1, half, 0, exs, vss)
                    emit_c_slices(p, KT // FG - 1, half, 1, exs, vss)
                    if p == 1:
                        for st in range(half * 8, half * 8 + 8):
                            emit_d_group(1, st, out1, tail=True)


    nc.compile()
    return nc


def _shard_inputs(input, Wqkv, bqkv, Wo):
    """Build the 8 per-core input dicts (host-side layout/sharding)."""
    bf16 = ml_dtypes.bfloat16
    in_maps = []
    for c in range(NCORES):
        b = c // 4
        g = c % 4
        heads = range(4 * g, 4 * g + 4)
        rows = (
            [slice(64 * h, 64 * h + 64) for h in heads]
            + [slice(E + 64 * h, E + 64 * h + 64) for h in heads]
            + [slice(2 * E + 64 * h, 2 * E + 64 * h + 64) for h in heads]
        )
        W_sel = np.concatenate([Wqkv[s] for s in rows], axis=0)  # [768, 1024]
        b_sel = np.concatenate([bqkv[s] for s in rows], axis=0)  # [768]
        in_maps.append(
            {
                "xT": np.ascontiguousarray(input[b].T).astype(bf16),
                "wT": np.ascontiguousarray(W_sel.T).astype(bf16),
                "bq": np.ascontiguousarray(b_sel[:QK].reshape(4, 128).T),
                "bv": np.ascontiguousarray(b_sel[QK:V3].reshape(1, 256)).astype(bf16),
                "woT": np.ascontiguousarray(
                    Wo[:, 4 * g * DH : 4 * (g + 1) * DH].T
                ).astype(bf16),
            }
        )
    return in_maps


def kernel(input, Wqkv, bqkv, Wo, bo, _trace=False):
    global LAST_RESULTS
    input = np.asarray(input, dtype=np.float32)
    Wqkv = np.asarray(Wqkv, dtype=np.float32)
    bqkv = np.asarray(bqkv, dtype=np.float32)
    Wo = np.asarray(Wo, dtype=np.float32)
    bo = np.asarray(bo, dtype=np.float32)

    nc = build_kernel()
    in_maps = _shard_inputs(input, Wqkv, bqkv, Wo)
    kwargs = {}
    if _trace:
        kwargs = dict(trace=True, trace_cores=[0])
    res = run_bass_kernel_spmd(nc, in_maps, core_ids=list(range(NCORES)), **kwargs)
    LAST_RESULTS = res

    out = np.zeros((B, S, E), dtype=np.float32)
    for c in range(NCORES):
        out[c // 4] += res.results[c]["out0"].astype(np.float32)
        out[c // 4] += res.results[c]["out1"].astype(np.float32)
    out += bo
    return out


# revision 6
# speedup vs baseline: 1.0981x; 1.0039x over previous
"""Multi-head attention (softmax over the QUERY axis) on 8 TRN2 NeuronCores.

Sharding: 2 batches x 4 head-groups (4 heads each) -> 8 cores.
Each core computes, for its (batch b, heads 4g..4g+3):
    qkT = W_{q,k} @ x_b^T + b_{q,k}   [512, 2048]   (e_out on partitions)
    V   = x_b @ W_v^T + b_v           [2048, 256]
    S'  = K Q^T (scores TRANSPOSED)   [k, q] per head
    P   = exp(S'/8) with fused row-sum -> denom[k]  (softmax over q == free dim)
    outT= sum_k (V[k,:]/denom[k]) P[k,:]            [d, q] per head
    part= outT^T @ WoT_g              [2048, 1024]  (partial for this head group)
Host sums the partials per batch (fp32) and adds bo.

Perf structure (v2):
  - A dense block of dummy PE matmuls at t=0 warms the HAM clock gate
    (PE 1.2 -> 2.4 GHz) while the input DMAs land, so all real matmuls
    run at the warm clock.
  - ACT (the only exp engine) is the roofline: 128 exp calls of
    [128,1024] + fused accumulator reads.  Everything else (PE matmul
    stream, DVE PSUM drains, DMA) is packed around that stream.
  - V is stored bf16 so the per-k V/denom scaling runs in DVE 4x mode.
  - Denominator bookkeeping (half-sum + reciprocal) is batched per
    4-ktile group into persistent tiles.
  - attn.V accumulates per 4-ktile group in PSUM; group flushes go
    straight into the SBUF fp32 accumulator, the last group's flush
    emits bf16 directly (no separate cast pass).
  - Projection outputs are written as bf16 partials; the p1 tail
    alternates its PSUM->SBUF copies between DVE and the idle ACT and
    interleaves the half0 projection with half1's attn.V tail.
"""

import sys

if "/opt/trn_rl_repo" not in sys.path:
    sys.path.insert(0, "/opt/trn_rl_repo")

import numpy as np
import ml_dtypes

import concourse.bass as bass
import concourse.mybir as mybir
import concourse.tile as tile
from concourse import bacc
from concourse.bass_utils import run_bass_kernel_spmd

F32 = mybir.dt.float32
BF16 = mybir.dt.bfloat16
AF = mybir.ActivationFunctionType

B, S, E, H = 2, 2048, 1024, 16
HL = 4  # heads per core
DH = 64
QK = 512  # q+k out dims per core (2*HL*DH)
V3 = 768  # q+k+v out dims per core
NCORES = 8

ET = E // 128  # 8 e-tiles
ST = S // 128  # 16 s-tiles
SC = S // 512  # 4 s/q chunks of 512
KT = ST  # 16 k-tiles
FG = 4  # k-tiles per attn.V accumulation group
NWARM = 12  # dummy matmuls to warm the PE clock gate

LAST_RESULTS = None


def build_kernel():
    nc = bacc.Bacc("TRN2", target_bir_lowering=False, debug=False, num_devices=NCORES)

    xT = nc.dram_tensor("xT", [E, S], BF16, kind="ExternalInput")
    wT = nc.dram_tensor("wT", [E, V3], BF16, kind="ExternalInput")
    bq = nc.dram_tensor("bq", [128, 4], F32, kind="ExternalInput")
    bv = nc.dram_tensor("bv", [1, 256], BF16, kind="ExternalInput")
    woT = nc.dram_tensor("woT", [2 * 128, E], BF16, kind="ExternalInput")
    out0 = nc.dram_tensor("out0", [S, E], BF16, kind="ExternalOutput")
    out1 = nc.dram_tensor("out1", [S, E], BF16, kind="ExternalOutput")

    with tile.TileContext(nc) as tc:
        with (
            tc.tile_pool(name="persist", bufs=1) as persist,
            tc.tile_pool(name="smalls", bufs=3) as smalls,
            tc.tile_pool(name="expp", bufs=2 * FG) as expp,
            tc.tile_pool(name="vsp", bufs=5) as vsp,
            tc.tile_pool(name="fout", bufs=2) as foutp,
            tc.tile_pool(name="mm_ps", bufs=2, space="PSUM") as mm_ps,
            tc.tile_pool(name="sp_ps", bufs=2, space="PSUM") as sp_ps,
            tc.tile_pool(name="ot_ps", bufs=1, space="PSUM") as ot_ps,
        ):
            qk_sb = persist.tile([128, 4, S], BF16, tag="qk")
            v_sb = persist.tile([128, ST, 256], BF16, tag="v")
            outT_f32 = persist.tile([128, 2, S], F32, tag="outT")
            outT_bf = persist.tile([128, 2, S], BF16, tag="outT_bf")
            bq_sb = persist.tile([128, 4], F32, tag="bq")
            bv_sb = persist.tile([1, 256], BF16, tag="bv")
            ones_sb = persist.tile([1, 512], BF16, tag="ones")
            den_sb = persist.tile([128, KT, 2, 2], F32, tag="den")
            xt_sb = persist.tile([128, ET, S], BF16, tag="xt")
            wt_sb = persist.tile([128, ET, V3], BF16, tag="wt")
            wo_sb = persist.tile([128, 2, E], BF16, tag="wo")

            # ---- PE warm-up: dense dummy matmuls while DMAs land ---------
            nc.vector.memset(ones_sb[:], 1.0)
            for _ in range(NWARM):
                wp = mm_ps.tile([128, 512], F32, tag="mmps")
                nc.tensor.matmul(
                    wp[:], ones_sb[0:1, 0:128], ones_sb[0:1, 0:512],
                    start=True, stop=True,
                )

            for et in range(ET):
                nc.gpsimd.dma_start(wt_sb[:, et, :], wT[et * 128 : (et + 1) * 128, :])
            for sc in range(SC):
                for et in range(ET):
                    nc.sync.dma_start(
                        xt_sb[:, et, sc * 512 : (sc + 1) * 512],
                        xT[et * 128 : (et + 1) * 128, sc * 512 : (sc + 1) * 512],
                    )
            nc.gpsimd.dma_start(bq_sb[:], bq[:])
            nc.gpsimd.dma_start(bv_sb[:], bv[:])
            for p in range(2):
                nc.gpsimd.dma_start(wo_sb[:, p, :], woT[p * 128 : (p + 1) * 128, :])

            # ---- emitters for qkT / V accumulation groups ----------------
            def emit_qk_group(eo, sc):
                pt = mm_ps.tile([128, 512], F32, tag="mmps")
                for et in range(ET):
                    nc.tensor.matmul(
                        pt[:],
                        wt_sb[:, et, eo * 128 : (eo + 1) * 128],
                        xt_sb[:, et, sc * 512 : (sc + 1) * 512],
                        start=(et == 0),
                        stop=(et == ET - 1),
                    )
                nc.vector.tensor_scalar_add(
                    qk_sb[:, eo, sc * 512 : (sc + 1) * 512],
                    in0=pt[:],
                    scalar1=bq_sb[:, eo : eo + 1],
                )

            def emit_v_group(st):
                pt = mm_ps.tile([128, 512], F32, tag="mmps")
                for et in range(ET):
                    nc.tensor.matmul(
                        pt[:, :256],
                        xt_sb[:, et, st * 128 : (st + 1) * 128],
                        wt_sb[:, et, QK:V3],
                        start=(et == 0),
                        stop=False,
                    )
                nc.tensor.matmul(  # + ones^T bv (bias row)
                    pt[:, :256],
                    ones_sb[0:1, 0:128],
                    bv_sb[0:1, :],
                    start=False,
                    stop=True,
                )
                nc.vector.tensor_copy(v_sb[:, st, :], pt[:, :256])

            def emit_d_group(p, st, out_dram, tail=False):
                ot = foutp.tile([128, E], BF16, tag="fout", name=f"fo_{p}_{st}")
                for nck in range(2):
                    pt = mm_ps.tile([128, 512], F32, tag="mmps", name=f"fp_{p}_{st}_{nck}")
                    nc.tensor.matmul(
                        pt[:],
                        outT_bf[:, p, st * 128 : (st + 1) * 128],
                        wo_sb[:, p, nck * 512 : (nck + 1) * 512],
                        start=True,
                        stop=True,
                    )
                    if tail and nck == 1:
                        nc.scalar.copy(ot[:, nck * 512 : (nck + 1) * 512], pt[:])
                    else:
                        nc.vector.tensor_copy(ot[:, nck * 512 : (nck + 1) * 512], pt[:])
                dma_eng = nc.sync if st % 2 == 0 else nc.gpsimd
                dma_eng.dma_start(out_dram[st * 128 : (st + 1) * 128, :], ot[:])

            # ---- pre-attention: just enough for pair0 kt0 ----------------
            # Emission order IS program order: every filler must be emitted
            # no later than the k-tile iteration that first consumes it
            # (fillers pop at the TOP of each k-tile iteration).
            emit_qk_group(0, 0)  # Q heads 0,1 cols 0-511
            emit_qk_group(0, 1)
            emit_qk_group(2, 0)  # K heads 0,1 cols 0-511 (kts 0-3)

            def qg(eo, sc):
                return lambda: emit_qk_group(eo, sc)

            def vg(st):
                return lambda: emit_v_group(st)

            fillers = (
                [vg(0), vg(1), qg(2, 1), vg(2), vg(3), qg(2, 2), vg(4), qg(2, 3)]
                + [vg(5), vg(6), vg(7), vg(8)]
                + [qg(1, 0), qg(1, 1), qg(1, 2), qg(1, 3)]
                + [vg(9), vg(10)]
                + [qg(3, 0), qg(3, 1)]
                + [vg(11), vg(12), vg(13), vg(14), vg(15)]
            )
            fillers.reverse()  # pop() from the front

            # ---- attention per head pair ---------------------------------
            # attn.V slices for group g are spread over group g+1's k-tiles
            # (2 of a half's 4 j-steps per k-tile) so the PE load per k-tile
            # is even and the exp stream never sees a burst.
            c_state = {}

            def emit_c_slices(p, g, half, jpair, exs, vss):
                if jpair == 0:
                    c_state[half] = ot_ps.tile(
                        [128, 1024], F32, tag="otps", name=f"oTt_{p}_{g}_{half}"
                    )
                oTt = c_state[half]
                for j in (2 * jpair, 2 * jpair + 1):
                    kt = FG * g + j
                    vs_g, jj = vss[kt]
                    for hh in range(2):
                        for qc in range(2):
                            q0 = half * 1024 + qc * 512
                            nc.tensor.matmul(
                                oTt[
                                    hh * 64 : (hh + 1) * 64,
                                    qc * 512 : (qc + 1) * 512,
                                ],
                                vs_g[:, jj, hh, :],
                                exs[kt][:, hh, q0 : q0 + 512],
                                start=(j == 0),
                                stop=(j == FG - 1),
                            )
                if jpair == 1:
                    f32dst = outT_f32[:, p, half * 1024 : (half + 1) * 1024]
                    if g == 0:
                        nc.vector.tensor_copy(f32dst, oTt[:])
                    elif g < KT // FG - 1:
                        nc.vector.tensor_add(f32dst, f32dst, oTt[:])
                    else:  # final group: emit bf16 directly
                        nc.vector.tensor_add(
                            outT_bf[:, p, half * 1024 : (half + 1) * 1024],
                            f32dst,
                            oTt[:],
                        )

            for p in range(2):
                exs = {}
                vss = {}

                def emit_scores_half(p, kt, half, ex):
                    for hh in range(2):
                        sp = sp_ps.tile([128, 1024], F32, tag="sp")
                        for qc in range(2):
                            q0 = half * 1024 + qc * 512
                            nc.tensor.matmul(
                                sp[:, qc * 512 : (qc + 1) * 512],
                                qk_sb[
                                    hh * 64 : (hh + 1) * 64,
                                    2 + p,
                                    kt * 128 : (kt + 1) * 128,
                                ],
                                qk_sb[hh * 64 : (hh + 1) * 64, p, q0 : q0 + 512],
                                start=True,
                                stop=True,
                            )
                        nc.scalar.activation(
                            ex[:, hh, half * 1024 : (half + 1) * 1024],
                            sp[:],
                            AF.Exp,
                            scale=0.125,
                            accum_out=den_sb[:, kt, hh, half : half + 1],
                        )

                for kt in range(KT):
                    ex = expp.tile([128, 2, S], BF16, tag="exp")
                    exs[kt] = ex
                    emit_scores_half(p, kt, 0, ex)
                    if p == 0 and kt == 0:
                        emit_qk_group(0, 2)  # Q cols 1024-2047 for half1
                        emit_qk_group(0, 3)
                    # previous group's attn.V between the two scores halves so
                    # the PE has queued work while ACT drains half0's exps
                    if kt >= FG:
                        o = kt % FG
                        emit_c_slices(p, kt // FG - 1, o // 2, o % 2, exs, vss)
                    emit_scores_half(p, kt, 1, ex)
                    # PE fillers (producers before their consumers)
                    if p == 0:
                        for _ in range(2):
                            if fillers:
                                fillers.pop()()
                    elif kt < 2:  # pair1 kt0/1: remaining K tiles for heads 2,3
                        emit_qk_group(3, 2 + kt)
                    else:  # pair1: overlap pair0's projection
                        emit_d_group(0, kt - 2, out0)
                        if kt >= 14:
                            emit_d_group(0, kt - 2 + 2, out0)
                    # batched denominator bookkeeping per 2-ktile pair
                    if kt % 2 == 1:
                        k0 = kt - 1
                        dsum = smalls.tile([128, 2, 2], F32, tag="dsum")
                        nc.vector.tensor_add(
                            dsum[:], den_sb[:, k0 : k0 + 2, :, 0], den_sb[:, k0 : k0 + 2, :, 1]
                        )
                        rec = smalls.tile([128, 2, 2], F32, tag="rec")
                        nc.vector.reciprocal(rec[:], dsum[:])
                        vs_g = vsp.tile([128, 2, 2, DH], BF16, tag="vs")
                        for j in range(2):
                            vss[k0 + j] = (vs_g, j)
                            for hh in range(2):
                                nc.vector.tensor_scalar_mul(
                                    vs_g[:, j, hh, :],
                                    in0=v_sb[:, k0 + j, (2 * p + hh) * 64 : (2 * p + hh + 1) * 64],
                                    scalar1=rec[:, j, hh : hh + 1],
                                )
                # tail: last group's attn.V (both q-halves) + flush; for p1
                # interleave the half0 projection with half1's attn.V tail
                for half in range(2):
                    emit_c_slices(p, KT // FG - 1, half, 0, exs, vss)
                    emit_c_slices(p, KT // FG - 1, half, 1, exs, vss)
                    if p == 1:
                        for st in range(half * 8, half * 8 + 8):
                            emit_d_group(1, st, out1, tail=True)


    nc.compile()
    return nc


def _shard_inputs(input, Wqkv, bqkv, Wo):
    """Build the 8 per-core input dicts (host-side layout/sharding)."""
    bf16 = ml_dtypes.bfloat16
    in_maps = []
    for c in range(NCORES):
        b = c // 4
        g = c % 4
        heads = range(4 * g, 4 * g + 4)
        rows = (
            [slice(64 * h, 64 * h + 64) for h in heads]
            + [slice(E + 64 * h, E + 64 * h + 64) for h in heads]
            + [slice(2 * E + 64 * h, 2 * E + 64 * h + 64) for h in heads]
        )
        W_sel = np.concatenate([Wqkv[s] for s in rows], axis=0)  # [768, 1024]
        b_sel = np.concatenate([bqkv[s] for s in rows], axis=0)  # [768]
        in_maps.append(
            {
                "xT": np.ascontiguousarray(input[b].T).astype(bf16),
                "wT": np.ascontiguousarray(W_sel.T).astype(bf16),
                "bq": np.ascontiguousarray(b_sel[:QK].reshape(4, 128).T),
                "bv": np.ascontiguousarray(b_sel[QK:V3].reshape(1, 256)).astype(bf16),
                "woT": np.ascontiguousarray(
                    Wo[:, 4 * g * DH : 4 * (g + 1) * DH].T
                ).astype(bf16),
            }
        )
    return in_maps


def kernel(input, Wqkv, bqkv, Wo, bo, _trace=False):
    global LAST_RESULTS
    input = np.asarray(input, dtype=np.float32)
    Wqkv = np.asarray(Wqkv, dtype=np.float32)
    bqkv = np.asarray(bqkv, dtype=np.float32)
    Wo = np.asarray(Wo, dtype=np.float32)
    bo = np.asarray(bo, dtype=np.float32)

    nc = build_kernel()
    in_maps = _shard_inputs(input, Wqkv, bqkv, Wo)
    kwargs = {}
    if _trace:
        kwargs = dict(trace=True, trace_cores=[0])
    res = run_bass_kernel_spmd(nc, in_maps, core_ids=list(range(NCORES)), **kwargs)
    LAST_RESULTS = res

    out = np.zeros((B, S, E), dtype=np.float32)
    for c in range(NCORES):
        out[c // 4] += res.results[c]["out0"].astype(np.float32)
        out[c // 4] += res.results[c]["out1"].astype(np.float32)
    out += bo
    return out


# revision 16
# speedup vs baseline: 1.1981x; 1.0911x over previous
"""Multi-head attention (softmax over the QUERY axis) on 8 TRN2 NeuronCores.

Sharding: 2 batches x 4 head-groups (4 heads each) -> 8 cores.
Each core computes, for its (batch b, heads 4g..4g+3):
    qkT = W_{q,k} @ x_b^T + b_{q,k}   [512, 2048]   (e_out on partitions)
    V   = x_b @ W_v^T + b_v           [2048, 256]
    S'  = K Q^T (scores TRANSPOSED)   [k, q] per head
    P   = exp(S'/8) with fused row-sum -> denom[k]  (softmax over q == free dim)
    outT= sum_k (V[k,:]/denom[k]) P[k,:]            [d, q] per head
    part= outT^T @ WoT_g              [2048, 1024]  (partial for this head group)
Host sums the partials per batch (fp32) and adds bo.

Perf structure (v2):
  - A dense block of dummy PE matmuls at t=0 warms the HAM clock gate
    (PE 1.2 -> 2.4 GHz) while the input DMAs land, so all real matmuls
    run at the warm clock.
  - ACT (the only exp engine) is the roofline: 128 exp calls of
    [128,1024] + fused accumulator reads.  Everything else (PE matmul
    stream, DVE PSUM drains, DMA) is packed around that stream.
  - V is stored bf16 so the per-k V/denom scaling runs in DVE 4x mode.
  - Denominator bookkeeping (half-sum + reciprocal) is batched per
    4-ktile group into persistent tiles.
  - attn.V accumulates per 4-ktile group in PSUM; group flushes go
    straight into the SBUF fp32 accumulator, the last group's flush
    emits bf16 directly (no separate cast pass).
  - Projection outputs are written as bf16 partials; the p1 tail
    alternates its PSUM->SBUF copies between DVE and the idle ACT and
    interleaves the half0 projection with half1's attn.V tail.
"""

import sys

if "/opt/trn_rl_repo" not in sys.path:
    sys.path.insert(0, "/opt/trn_rl_repo")

import numpy as np
import ml_dtypes

import concourse.bass as bass
import concourse.mybir as mybir
import concourse.tile as tile
from concourse import bacc
from concourse.bass_utils import run_bass_kernel_spmd

F32 = mybir.dt.float32
F16 = mybir.dt.float16
BF16 = mybir.dt.bfloat16
AF = mybir.ActivationFunctionType

B, S, E, H = 2, 2048, 1024, 16
HL = 4  # heads per core
DH = 64
QK = 512  # q+k out dims per core (2*HL*DH)
V3 = 768  # q+k+v out dims per core
NCORES = 8

ET = E // 128  # 8 e-tiles
ST = S // 128  # 16 s-tiles
SC = S // 512  # 4 s/q chunks of 512
KT = ST  # 16 k-tiles
FG = 4  # k-tiles per attn.V accumulation group
NWARM = 12  # dummy matmuls to warm the PE clock gate

LAST_RESULTS = None


def build_kernel():
    nc = bacc.Bacc("TRN2", target_bir_lowering=False, debug=False, num_devices=NCORES)

    xT = nc.dram_tensor("xT", [E, S], BF16, kind="ExternalInput")
    wT = nc.dram_tensor("wT", [E, V3], BF16, kind="ExternalInput")
    bq = nc.dram_tensor("bq", [128, 4], F32, kind="ExternalInput")
    bv = nc.dram_tensor("bv", [1, 256], BF16, kind="ExternalInput")
    woT = nc.dram_tensor("woT", [2 * 128, E], BF16, kind="ExternalInput")
    out0 = nc.dram_tensor("out0", [S, E], BF16, kind="ExternalOutput")
    out1 = nc.dram_tensor("out1", [S, E], BF16, kind="ExternalOutput")

    with tile.TileContext(nc) as tc:
        with (
            tc.tile_pool(name="persist", bufs=1) as persist,
            tc.tile_pool(name="smalls", bufs=3) as smalls,
            tc.tile_pool(name="expp", bufs=2 * FG) as expp,
            tc.tile_pool(name="vsp", bufs=5) as vsp,
            tc.tile_pool(name="fout", bufs=2) as foutp,
            tc.tile_pool(name="mm_ps", bufs=2, space="PSUM") as mm_ps,
            tc.tile_pool(name="sp_ps", bufs=2, space="PSUM") as sp_ps,
            tc.tile_pool(name="ot_ps", bufs=1, space="PSUM") as ot_ps,
        ):
            qk_sb = persist.tile([128, 4, S], BF16, tag="qk")
            v_sb = persist.tile([128, ST, 256], BF16, tag="v")
            outT_f32 = persist.tile([128, 2, S], F32, tag="outT")
            outT_bf = persist.tile([128, 2, S], BF16, tag="outT_bf")
            bq_sb = persist.tile([128, 4], F32, tag="bq")
            bv_sb = persist.tile([1, 256], BF16, tag="bv")
            ones_sb = persist.tile([1, 512], BF16, tag="ones")
            den_sb = persist.tile([128, KT, 2, 2], F32, tag="den")
            xt_sb = persist.tile([128, ET, S], BF16, tag="xt")
            wt_sb = persist.tile([128, ET, V3], BF16, tag="wt")
            wo_sb = persist.tile([128, 2, E], BF16, tag="wo")

            # ---- PE warm-up: dense dummy matmuls while DMAs land ---------
            nc.vector.memset(ones_sb[:], 1.0)
            for _ in range(NWARM):
                wp = mm_ps.tile([128, 512], F32, tag="mmps")
                nc.tensor.matmul(
                    wp[:], ones_sb[0:1, 0:128], ones_sb[0:1, 0:512],
                    start=True, stop=True,
                )

            # batched input DMAs: one 1.5MB load for wT (SWDGE ring), one 1MB
            # load per 512-column chunk of xT alternating across the two
            # HWDGE rings (SP + ACT), so the qk chain unblocks ASAP
            nc.gpsimd.dma_start(
                wt_sb[:], wT[:, :].rearrange("(et p) v -> p et v", p=128)
            )
            for sc in range(SC):
                dma_eng = nc.sync if sc % 2 == 0 else nc.scalar
                dma_eng.dma_start(
                    xt_sb[:, :, sc * 512 : (sc + 1) * 512],
                    xT[:, sc * 512 : (sc + 1) * 512].rearrange(
                        "(et p) s -> p et s", p=128
                    ),
                )
            nc.gpsimd.dma_start(bq_sb[:], bq[:])
            nc.gpsimd.dma_start(bv_sb[:], bv[:])
            for p in range(2):
                nc.gpsimd.dma_start(wo_sb[:, p, :], woT[p * 128 : (p + 1) * 128, :])

            # ---- emitters for qkT / V accumulation groups ----------------
            def emit_qk_group(eo, sc):
                pt = mm_ps.tile([128, 512], F32, tag="mmps")
                for et in range(ET):
                    nc.tensor.matmul(
                        pt[:],
                        wt_sb[:, et, eo * 128 : (eo + 1) * 128],
                        xt_sb[:, et, sc * 512 : (sc + 1) * 512],
                        start=(et == 0),
                        stop=(et == ET - 1),
                    )
                nc.vector.tensor_scalar_add(
                    qk_sb[:, eo, sc * 512 : (sc + 1) * 512],
                    in0=pt[:],
                    scalar1=bq_sb[:, eo : eo + 1],
                )

            def emit_v_group(st):
                pt = mm_ps.tile([128, 512], F32, tag="mmps")
                for et in range(ET):
                    nc.tensor.matmul(
                        pt[:, :256],
                        xt_sb[:, et, st * 128 : (st + 1) * 128],
                        wt_sb[:, et, QK:V3],
                        start=(et == 0),
                        stop=False,
                    )
                nc.tensor.matmul(  # + ones^T bv (bias row)
                    pt[:, :256],
                    ones_sb[0:1, 0:128],
                    bv_sb[0:1, :],
                    start=False,
                    stop=True,
                )
                nc.vector.tensor_copy(v_sb[:, st, :], pt[:, :256])

            def emit_d_group(p, st, out_dram, tail=False):
                ot = foutp.tile([128, E], BF16, tag="fout", name=f"fo_{p}_{st}")
                for nck in range(2):
                    pt = mm_ps.tile([128, 512], F32, tag="mmps", name=f"fp_{p}_{st}_{nck}")
                    nc.tensor.matmul(
                        pt[:],
                        outT_bf[:, p, st * 128 : (st + 1) * 128],
                        wo_sb[:, p, nck * 512 : (nck + 1) * 512],
                        start=True,
                        stop=True,
                    )
                    if tail and nck == 1:
                        nc.scalar.copy(ot[:, nck * 512 : (nck + 1) * 512], pt[:])
                    else:
                        nc.vector.tensor_copy(ot[:, nck * 512 : (nck + 1) * 512], pt[:])
                dma_eng = nc.sync if st % 2 == 0 else nc.gpsimd
                dma_eng.dma_start(out_dram[st * 128 : (st + 1) * 128, :], ot[:])

            # ---- pre-attention: just enough for pair0 kt0 ----------------
            # Emission order IS program order: every filler must be emitted
            # no later than the k-tile iteration that first consumes it
            # (fillers pop at the TOP of each k-tile iteration).
            emit_qk_group(0, 0)  # Q heads 0,1 cols 0-511
            emit_qk_group(0, 1)
            emit_qk_group(2, 0)  # K heads 0,1 cols 0-511 (kts 0-3)

            def qg(eo, sc):
                return lambda: emit_qk_group(eo, sc)

            def vg(st):
                return lambda: emit_v_group(st)

            fillers = (
                [vg(0), vg(1), qg(2, 1), vg(2), vg(3), qg(2, 2), vg(4), qg(2, 3)]
                + [vg(5), vg(6), vg(7), vg(8)]
                + [qg(1, 0), qg(1, 1), qg(1, 2), qg(1, 3)]
                + [vg(9), vg(10)]
                + [qg(3, 0), qg(3, 1)]
                + [vg(11), vg(12), vg(13), vg(14), vg(15)]
            )
            fillers.reverse()  # pop() from the front

            # ---- attention per head pair ---------------------------------
            # attn.V slices for group g are spread over group g+1's k-tiles
            # (2 of a half's 4 j-steps per k-tile) so the PE load per k-tile
            # is even and the exp stream never sees a burst.
            c_state = {}

            def emit_c_slices(p, g, half, jpair, exs, vss):
                if jpair == 0:
                    c_state[half] = ot_ps.tile(
                        [128, 1024], F32, tag="otps", name=f"oTt_{p}_{g}_{half}"
                    )
                oTt = c_state[half]
                for j in (2 * jpair, 2 * jpair + 1):
                    kt = FG * g + j
                    vs_g, jj = vss[kt]
                    for qc in range(2):
                        for hh in range(2):  # hh-adjacent: disjoint col groups
                            q0 = half * 1024 + qc * 512
                            nc.tensor.matmul(
                                oTt[
                                    hh * 64 : (hh + 1) * 64,
                                    qc * 512 : (qc + 1) * 512,
                                ],
                                vs_g[:, jj, hh, :],
                                exs[kt][:, hh, q0 : q0 + 512],
                                start=(j == 0),
                                stop=(j == FG - 1),
                            )
                if jpair == 1:
                    f32dst = outT_f32[:, p, half * 1024 : (half + 1) * 1024]
                    if g == 0:
                        nc.vector.tensor_copy(f32dst, oTt[:])
                    elif g < KT // FG - 1:
                        nc.vector.tensor_add(f32dst, f32dst, oTt[:])
                    else:  # final group: emit bf16 directly
                        nc.vector.tensor_add(
                            outT_bf[:, p, half * 1024 : (half + 1) * 1024],
                            f32dst,
                            oTt[:],
                        )

            for p in range(2):
                exs = {}
                vss = {}

                def emit_scores_half(p, kt, half, ex):
                    # two fp32 PSUM tiles (one per head); matmuls interleaved
                    # hh-adjacent so consecutive MMs hit disjoint stationary
                    # row groups (PE row-tiling concurrency)
                    sps = [
                        sp_ps.tile([128, 1024], F32, tag="sp", name=f"sp{p}_{kt}_{half}_{hh}")
                        for hh in range(2)
                    ]
                    for qc in range(2):
                        for hh in range(2):
                            q0 = half * 1024 + qc * 512
                            nc.tensor.matmul(
                                sps[hh][:, qc * 512 : (qc + 1) * 512],
                                qk_sb[
                                    hh * 64 : (hh + 1) * 64,
                                    2 + p,
                                    kt * 128 : (kt + 1) * 128,
                                ],
                                qk_sb[hh * 64 : (hh + 1) * 64, p, q0 : q0 + 512],
                                start=True,
                                stop=True,
                            )
                    for hh in range(2):
                        nc.scalar.activation(
                            ex[:, hh, half * 1024 : (half + 1) * 1024],
                            sps[hh][:],
                            AF.Exp,
                            scale=0.125,
                            accum_out=den_sb[:, kt, hh, half : half + 1],
                        )

                for kt in range(KT):
                    ex = expp.tile([128, 2, S], BF16, tag="exp")
                    exs[kt] = ex
                    emit_scores_half(p, kt, 0, ex)
                    if p == 0 and kt == 0:
                        emit_qk_group(0, 2)  # Q cols 1024-2047 for half1
                        emit_qk_group(0, 3)
                    # previous group's attn.V between the two scores halves so
                    # the PE has queued work while ACT drains half0's exps
                    if kt >= FG:
                        o = kt % FG
                        emit_c_slices(p, kt // FG - 1, o // 2, o % 2, exs, vss)
                    emit_scores_half(p, kt, 1, ex)
                    # PE fillers (producers before their consumers)
                    if p == 0:
                        for _ in range(2):
                            if fillers:
                                fillers.pop()()
                    elif kt < 2:  # pair1 kt0/1: remaining K tiles for heads 2,3
                        emit_qk_group(3, 2 + kt)
                    else:  # pair1: overlap pair0's projection
                        emit_d_group(0, kt - 2, out0)
                        if kt >= 14:
                            emit_d_group(0, kt - 2 + 2, out0)
                    # batched denominator bookkeeping per 2-ktile pair
                    if kt % 2 == 1:
                        k0 = kt - 1
                        dsum = smalls.tile([128, 2, 2], F32, tag="dsum")
                        nc.vector.tensor_add(
                            dsum[:],
                            den_sb[:, k0 : k0 + 2, :, 0],
                            den_sb[:, k0 : k0 + 2, :, 1],
                        )
                        rec = smalls.tile([128, 2, 2], F32, tag="rec")
                        nc.vector.reciprocal(rec[:], dsum[:])
                        vs_g = vsp.tile([128, 2, 2, DH], BF16, tag="vs")
                        for j in range(2):
                            vss[k0 + j] = (vs_g, j)
                            for hh in range(2):
                                nc.vector.tensor_scalar_mul(
                                    vs_g[:, j, hh, :],
                                    in0=v_sb[:, k0 + j, (2 * p + hh) * 64 : (2 * p + hh + 1) * 64],
                                    scalar1=rec[:, j, hh : hh + 1],
                                )
                # tail: last group's attn.V (both q-halves) + flush; for p1
                # interleave the half0 projection with half1's attn.V tail
                for half in range(2):
                    emit_c_slices(p, KT // FG - 1, half, 0, exs, vss)
                    emit_c_slices(p, KT // FG - 1, half, 1, exs, vss)
                    if p == 1:
                        for st in range(half * 8, half * 8 + 8):
                            emit_d_group(1, st, out1, tail=True)


    nc.compile()
    return nc


def _shard_inputs(input, Wqkv, bqkv, Wo):
    """Build the 8 per-core input dicts (host-side layout/sharding)."""
    bf16 = ml_dtypes.bfloat16
    in_maps = []
    for c in range(NCORES):
        b = c // 4
        g = c % 4
        heads = range(4 * g, 4 * g + 4)
        rows = (
            [slice(64 * h, 64 * h + 64) for h in heads]
            + [slice(E + 64 * h, E + 64 * h + 64) for h in heads]
            + [slice(2 * E + 64 * h, 2 * E + 64 * h + 64) for h in heads]
        )
        W_sel = np.concatenate([Wqkv[s] for s in rows], axis=0)  # [768, 1024]
        b_sel = np.concatenate([bqkv[s] for s in rows], axis=0)  # [768]
        in_maps.append(
            {
                "xT": np.ascontiguousarray(input[b].T).astype(bf16),
                "wT": np.ascontiguousarray(W_sel.T).astype(bf16),
                "bq": np.ascontiguousarray(b_sel[:QK].reshape(4, 128).T),
                "bv": np.ascontiguousarray(b_sel[QK:V3].reshape(1, 256)).astype(bf16),
                "woT": np.ascontiguousarray(
                    Wo[:, 4 * g * DH : 4 * (g + 1) * DH].T
                ).astype(bf16),
            }
        )
    return in_maps


def kernel(input, Wqkv, bqkv, Wo, bo, _trace=False):
    global LAST_RESULTS
    input = np.asarray(input, dtype=np.float32)
    Wqkv = np.asarray(Wqkv, dtype=np.float32)
    bqkv = np.asarray(bqkv, dtype=np.float32)
    Wo = np.asarray(Wo, dtype=np.float32)
    bo = np.asarray(bo, dtype=np.float32)

    nc = build_kernel()
    in_maps = _shard_inputs(input, Wqkv, bqkv, Wo)
    kwargs = {}
    if _trace:
        kwargs = dict(trace=True, trace_cores=[0])
    res = run_bass_kernel_spmd(nc, in_maps, core_ids=list(range(NCORES)), **kwargs)
    LAST_RESULTS = res

    out = np.zeros((B, S, E), dtype=np.float32)
    for c in range(NCORES):
        out[c // 4] += res.results[c]["out0"].astype(np.float32)
        out[c // 4] += res.results[c]["out1"].astype(np.float32)
    out += bo
    return out


# revision 19
# speedup vs baseline: 1.2243x; 1.0218x over previous
"""Multi-head attention (softmax over the QUERY axis) on 8 TRN2 NeuronCores.

Sharding: 2 batches x 4 head-groups (4 heads each) -> 8 cores.
Each core computes, for its (batch b, heads 4g..4g+3):
    qkT = W_{q,k} @ x_b^T + b_{q,k}   [512, 2048]   (e_out on partitions)
    V   = x_b @ W_v^T + b_v           [2048, 256]
    S'  = K Q^T (scores TRANSPOSED)   [k, q] per head
    P   = exp(S'/8) with fused row-sum -> denom[k]  (softmax over q == free dim)
    outT= sum_k (V[k,:]/denom[k]) P[k,:]            [d, q] per head
    part= outT^T @ WoT_g              [2048, 1024]  (partial for this head group)
Host sums the partials per batch (fp32) and adds bo.

Perf structure (v2):
  - A dense block of dummy PE matmuls at t=0 warms the HAM clock gate
    (PE 1.2 -> 2.4 GHz) while the input DMAs land, so all real matmuls
    run at the warm clock.
  - ACT (the only exp engine) is the roofline: 128 exp calls of
    [128,1024] + fused accumulator reads.  Everything else (PE matmul
    stream, DVE PSUM drains, DMA) is packed around that stream.
  - V is stored bf16 so the per-k V/denom scaling runs in DVE 4x mode.
  - Denominator bookkeeping (half-sum + reciprocal) is batched per
    4-ktile group into persistent tiles.
  - attn.V accumulates per 4-ktile group in PSUM; group flushes go
    straight into the SBUF fp32 accumulator, the last group's flush
    emits bf16 directly (no separate cast pass).
  - Projection outputs are written as bf16 partials; the p1 tail
    alternates its PSUM->SBUF copies between DVE and the idle ACT and
    interleaves the half0 projection with half1's attn.V tail.
"""

import sys

if "/opt/trn_rl_repo" not in sys.path:
    sys.path.insert(0, "/opt/trn_rl_repo")

import numpy as np
import ml_dtypes

import concourse.bass as bass
import concourse.mybir as mybir
import concourse.tile as tile
from concourse import bacc
from concourse.bass_utils import run_bass_kernel_spmd

F32 = mybir.dt.float32
F16 = mybir.dt.float16
BF16 = mybir.dt.bfloat16
AF = mybir.ActivationFunctionType

B, S, E, H = 2, 2048, 1024, 16
HL = 4  # heads per core
DH = 64
QK = 512  # q+k out dims per core (2*HL*DH)
V3 = 768  # q+k+v out dims per core
NCORES = 8

ET = E // 128  # 8 e-tiles
ST = S // 128  # 16 s-tiles
SC = S // 512  # 4 s/q chunks of 512
KT = ST  # 16 k-tiles
FG = 4  # k-tiles per attn.V accumulation group
NWARM = 12  # dummy matmuls to warm the PE clock gate

LAST_RESULTS = None


def build_kernel():
    nc = bacc.Bacc("TRN2", target_bir_lowering=False, debug=False, num_devices=NCORES)

    xT = nc.dram_tensor("xT", [E, S], BF16, kind="ExternalInput")
    wT = nc.dram_tensor("wT", [E, V3], BF16, kind="ExternalInput")
    bq = nc.dram_tensor("bq", [128, 4], F32, kind="ExternalInput")
    bv = nc.dram_tensor("bv", [1, 256], BF16, kind="ExternalInput")
    woT = nc.dram_tensor("woT", [2 * 128, E], BF16, kind="ExternalInput")
    out0 = nc.dram_tensor("out0", [S, E], BF16, kind="ExternalOutput")
    out1 = nc.dram_tensor("out1", [S, E], BF16, kind="ExternalOutput")

    with tile.TileContext(nc) as tc:
        with (
            tc.tile_pool(name="persist", bufs=1) as persist,
            tc.tile_pool(name="smalls", bufs=3) as smalls,
            tc.tile_pool(name="expp", bufs=2 * FG) as expp,
            tc.tile_pool(name="vsp", bufs=5) as vsp,
            tc.tile_pool(name="fout", bufs=4) as foutp,
            tc.tile_pool(name="mm_ps", bufs=2, space="PSUM") as mm_ps,
            tc.tile_pool(name="sp_ps", bufs=2, space="PSUM") as sp_ps,
            tc.tile_pool(name="ot_ps", bufs=1, space="PSUM") as ot_ps,
        ):
            qk_sb = persist.tile([128, 4, S], BF16, tag="qk")
            v_sb = persist.tile([128, ST, 256], BF16, tag="v")
            outT_f32 = persist.tile([128, 2, S], F32, tag="outT")
            outT_bf = persist.tile([128, 2, S], BF16, tag="outT_bf")
            bq_sb = persist.tile([128, 4], F32, tag="bq")
            bv_sb = persist.tile([1, 256], BF16, tag="bv")
            ones_sb = persist.tile([1, 512], BF16, tag="ones")
            den_sb = persist.tile([128, KT, 2, 2], F32, tag="den")
            xt_sb = persist.tile([128, ET, S], BF16, tag="xt")
            wt_sb = persist.tile([128, ET, V3], BF16, tag="wt")
            wo_sb = persist.tile([128, 2, E], BF16, tag="wo")

            # ---- PE warm-up: dense dummy matmuls while DMAs land ---------
            nc.vector.memset(ones_sb[:], 1.0)
            for _ in range(NWARM):
                wp = mm_ps.tile([128, 512], F32, tag="mmps")
                nc.tensor.matmul(
                    wp[:], ones_sb[0:1, 0:128], ones_sb[0:1, 0:512],
                    start=True, stop=True,
                )

            # chunked input DMAs so the first qk matmuls can trickle-fire as
            # each (et) chunk lands: wT on the SWDGE ring, xT split across
            # the two HWDGE rings (SP + ACT) in sc-major order
            for et in range(ET):
                nc.gpsimd.dma_start(wt_sb[:, et, :], wT[et * 128 : (et + 1) * 128, :])
            for sc in range(SC):
                dma_eng = nc.sync if sc % 2 == 0 else nc.scalar
                for et in range(ET):
                    dma_eng.dma_start(
                        xt_sb[:, et, sc * 512 : (sc + 1) * 512],
                        xT[et * 128 : (et + 1) * 128, sc * 512 : (sc + 1) * 512],
                    )
            nc.gpsimd.dma_start(bq_sb[:], bq[:])
            nc.gpsimd.dma_start(bv_sb[:], bv[:])
            for p in range(2):
                nc.gpsimd.dma_start(wo_sb[:, p, :], woT[p * 128 : (p + 1) * 128, :])

            # ---- emitters for qkT / V accumulation groups ----------------
            def emit_qk_group(eo, sc):
                pt = mm_ps.tile([128, 512], F32, tag="mmps")
                for et in range(ET):
                    nc.tensor.matmul(
                        pt[:],
                        wt_sb[:, et, eo * 128 : (eo + 1) * 128],
                        xt_sb[:, et, sc * 512 : (sc + 1) * 512],
                        start=(et == 0),
                        stop=(et == ET - 1),
                    )
                nc.vector.tensor_scalar_add(
                    qk_sb[:, eo, sc * 512 : (sc + 1) * 512],
                    in0=pt[:],
                    scalar1=bq_sb[:, eo : eo + 1],
                )

            def emit_v_group(st):
                pt = mm_ps.tile([128, 512], F32, tag="mmps")
                for et in range(ET):
                    nc.tensor.matmul(
                        pt[:, :256],
                        xt_sb[:, et, st * 128 : (st + 1) * 128],
                        wt_sb[:, et, QK:V3],
                        start=(et == 0),
                        stop=False,
                    )
                nc.tensor.matmul(  # + ones^T bv (bias row)
                    pt[:, :256],
                    ones_sb[0:1, 0:128],
                    bv_sb[0:1, :],
                    start=False,
                    stop=True,
                )
                nc.vector.tensor_copy(v_sb[:, st, :], pt[:, :256])

            def emit_d_group(p, st, out_dram, tail=False):
                ot = foutp.tile([128, E], BF16, tag="fout", name=f"fo_{p}_{st}")
                for nck in range(2):
                    pt = mm_ps.tile([128, 512], F32, tag="mmps", name=f"fp_{p}_{st}_{nck}")
                    nc.tensor.matmul(
                        pt[:],
                        outT_bf[:, p, st * 128 : (st + 1) * 128],
                        wo_sb[:, p, nck * 512 : (nck + 1) * 512],
                        start=True,
                        stop=True,
                    )
                    if tail and nck == 1:
                        nc.scalar.copy(ot[:, nck * 512 : (nck + 1) * 512], pt[:])
                    else:
                        nc.vector.tensor_copy(ot[:, nck * 512 : (nck + 1) * 512], pt[:])
                dma_eng = nc.sync if st % 2 == 0 else nc.gpsimd
                dma_eng.dma_start(out_dram[st * 128 : (st + 1) * 128, :], ot[:])

            # ---- pre-attention: just enough for pair0 kt0 ----------------
            # Emission order IS program order: every filler must be emitted
            # no later than the k-tile iteration that first consumes it
            # (fillers pop at the TOP of each k-tile iteration).
            emit_qk_group(0, 0)  # Q heads 0,1 cols 0-511
            emit_qk_group(0, 1)
            emit_qk_group(2, 0)  # K heads 0,1 cols 0-511 (kts 0-3)

            def qg(eo, sc):
                return lambda: emit_qk_group(eo, sc)

            def vg(st):
                return lambda: emit_v_group(st)

            fillers = (
                [vg(0), vg(1), qg(2, 1), vg(2), vg(3), qg(2, 2), vg(4), qg(2, 3)]
                + [vg(5), vg(6), vg(7), vg(8)]
                + [qg(1, 0), qg(1, 1), qg(1, 2), qg(1, 3)]
                + [vg(9), vg(10)]
                + [qg(3, 0), qg(3, 1)]
                + [vg(11), vg(12), vg(13), vg(14), vg(15)]
            )
            fillers.reverse()  # pop() from the front

            # ---- attention per head pair ---------------------------------
            # attn.V slices for group g are spread over group g+1's k-tiles
            # (2 of a half's 4 j-steps per k-tile) so the PE load per k-tile
            # is even and the exp stream never sees a burst.
            c_state = {}

            def emit_c_slices(p, g, half, jpair, exs, vss):
                if jpair == 0:
                    c_state[half] = ot_ps.tile(
                        [128, 1024], F32, tag="otps", name=f"oTt_{p}_{g}_{half}"
                    )
                oTt = c_state[half]
                for j in (2 * jpair, 2 * jpair + 1):
                    kt = FG * g + j
                    vs_g, jj = vss[kt]
                    for qc in range(2):
                        for hh in range(2):  # hh-adjacent: disjoint col groups
                            q0 = half * 1024 + qc * 512
                            nc.tensor.matmul(
                                oTt[
                                    hh * 64 : (hh + 1) * 64,
                                    qc * 512 : (qc + 1) * 512,
                                ],
                                vs_g[:, jj, hh, :],
                                exs[kt][:, hh, q0 : q0 + 512],
                                start=(j == 0),
                                stop=(j == FG - 1),
                            )
                if jpair == 1:
                    f32dst = outT_f32[:, p, half * 1024 : (half + 1) * 1024]
                    if g == 0:
                        nc.vector.tensor_copy(f32dst, oTt[:])
                    elif g < KT // FG - 1:
                        nc.vector.tensor_add(f32dst, f32dst, oTt[:])
                    else:  # final group: emit bf16 directly
                        nc.vector.tensor_add(
                            outT_bf[:, p, half * 1024 : (half + 1) * 1024],
                            f32dst,
                            oTt[:],
                        )

            for p in range(2):
                exs = {}
                vss = {}

                def emit_scores_half(p, kt, half, ex):
                    # two fp32 PSUM tiles (one per head); matmuls interleaved
                    # hh-adjacent so consecutive MMs hit disjoint stationary
                    # row groups (PE row-tiling concurrency)
                    sps = [
                        sp_ps.tile([128, 1024], F32, tag="sp", name=f"sp{p}_{kt}_{half}_{hh}")
                        for hh in range(2)
                    ]
                    for qc in range(2):
                        for hh in range(2):
                            q0 = half * 1024 + qc * 512
                            nc.tensor.matmul(
                                sps[hh][:, qc * 512 : (qc + 1) * 512],
                                qk_sb[
                                    hh * 64 : (hh + 1) * 64,
                                    2 + p,
                                    kt * 128 : (kt + 1) * 128,
                                ],
                                qk_sb[hh * 64 : (hh + 1) * 64, p, q0 : q0 + 512],
                                start=True,
                                stop=True,
                            )
                    for hh in range(2):
                        nc.scalar.activation(
                            ex[:, hh, half * 1024 : (half + 1) * 1024],
                            sps[hh][:],
                            AF.Exp,
                            scale=0.125,
                            accum_out=den_sb[:, kt, hh, half : half + 1],
                        )

                for kt in range(KT):
                    ex = expp.tile([128, 2, S], BF16, tag="exp")
                    exs[kt] = ex
                    emit_scores_half(p, kt, 0, ex)
                    if p == 0 and kt == 0:
                        emit_qk_group(0, 2)  # Q cols 1024-2047 for half1
                        emit_qk_group(0, 3)
                    # previous group's attn.V between the two scores halves so
                    # the PE has queued work while ACT drains half0's exps
                    if kt >= FG:
                        o = kt % FG
                        emit_c_slices(p, kt // FG - 1, o // 2, o % 2, exs, vss)
                    emit_scores_half(p, kt, 1, ex)
                    # PE fillers (producers before their consumers)
                    if p == 0:
                        for _ in range(2):
                            if fillers:
                                fillers.pop()()
                    elif kt < 2:  # pair1 kt0/1: remaining K tiles for heads 2,3
                        emit_qk_group(3, 2 + kt)
                    else:  # pair1: overlap pair0's projection
                        emit_d_group(0, kt - 2, out0)
                        if kt >= 14:
                            emit_d_group(0, kt - 2 + 2, out0)
                    # batched denominator bookkeeping per 2-ktile pair
                    if kt % 2 == 1:
                        k0 = kt - 1
                        dsum = smalls.tile([128, 2, 2], F32, tag="dsum")
                        nc.vector.tensor_add(
                            dsum[:],
                            den_sb[:, k0 : k0 + 2, :, 0],
                            den_sb[:, k0 : k0 + 2, :, 1],
                        )
                        rec = smalls.tile([128, 2, 2], F32, tag="rec")
                        nc.vector.reciprocal(rec[:], dsum[:])
                        vs_g = vsp.tile([128, 2, 2, DH], BF16, tag="vs")
                        for j in range(2):
                            vss[k0 + j] = (vs_g, j)
                            for hh in range(2):
                                nc.vector.tensor_scalar_mul(
                                    vs_g[:, j, hh, :],
                                    in0=v_sb[:, k0 + j, (2 * p + hh) * 64 : (2 * p + hh + 1) * 64],
                                    scalar1=rec[:, j, hh : hh + 1],
                                )
                # tail: last group's attn.V (both q-halves) + flush; for p1
                # interleave the half0 projection with half1's attn.V tail
                gl = KT // FG - 1
                emit_c_slices(p, gl, 0, 0, exs, vss)
                emit_c_slices(p, gl, 0, 1, exs, vss)
                if p == 0:
                    emit_c_slices(p, gl, 1, 0, exs, vss)
                    emit_c_slices(p, gl, 1, 1, exs, vss)
                else:
                    emit_d_group(1, 0, out1, tail=True)
                    emit_d_group(1, 1, out1, tail=True)
                    emit_c_slices(p, gl, 1, 0, exs, vss)
                    emit_d_group(1, 2, out1, tail=True)
                    emit_d_group(1, 3, out1, tail=True)
                    emit_c_slices(p, gl, 1, 1, exs, vss)
                    for st in range(4, ST):
                        emit_d_group(1, st, out1, tail=True)


    nc.compile()
    return nc


def _shard_inputs(input, Wqkv, bqkv, Wo):
    """Build the 8 per-core input dicts (host-side layout/sharding)."""
    bf16 = ml_dtypes.bfloat16
    in_maps = []
    for c in range(NCORES):
        b = c // 4
        g = c % 4
        heads = range(4 * g, 4 * g + 4)
        rows = (
            [slice(64 * h, 64 * h + 64) for h in heads]
            + [slice(E + 64 * h, E + 64 * h + 64) for h in heads]
            + [slice(2 * E + 64 * h, 2 * E + 64 * h + 64) for h in heads]
        )
        W_sel = np.concatenate([Wqkv[s] for s in rows], axis=0)  # [768, 1024]
        b_sel = np.concatenate([bqkv[s] for s in rows], axis=0)  # [768]
        in_maps.append(
            {
                "xT": np.ascontiguousarray(input[b].T).astype(bf16),
                "wT": np.ascontiguousarray(W_sel.T).astype(bf16),
                "bq": np.ascontiguousarray(b_sel[:QK].reshape(4, 128).T),
                "bv": np.ascontiguousarray(b_sel[QK:V3].reshape(1, 256)).astype(bf16),
                "woT": np.ascontiguousarray(
                    Wo[:, 4 * g * DH : 4 * (g + 1) * DH].T
                ).astype(bf16),
            }
        )
    return in_maps


def kernel(input, Wqkv, bqkv, Wo, bo, _trace=False):
    global LAST_RESULTS
    input = np.asarray(input, dtype=np.float32)
    Wqkv = np.asarray(Wqkv, dtype=np.float32)
    bqkv = np.asarray(bqkv, dtype=np.float32)
    Wo = np.asarray(Wo, dtype=np.float32)
    bo = np.asarray(bo, dtype=np.float32)

    nc = build_kernel()
    in_maps = _shard_inputs(input, Wqkv, bqkv, Wo)
    kwargs = {}
    if _trace:
        kwargs = dict(trace=True, trace_cores=[0])
    res = run_bass_kernel_spmd(nc, in_maps, core_ids=list(range(NCORES)), **kwargs)
    LAST_RESULTS = res

    out = np.zeros((B, S, E), dtype=np.float32)
    for c in range(NCORES):
        out[c // 4] += res.results[c]["out0"].astype(np.float32)
        out[c // 4] += res.results[c]["out1"].astype(np.float32)
    out += bo
    return out
